# revision 54
# baseline (speedup 1.0000x reference)
"""Multi-head attention (B=2, QL=KL=2048, DIM=1024, H=16) on 8 TRN2 NeuronCores.

Sharding: core c handles batch c//4 and heads (c%4)*4 .. (c%4)*4+4 (column-
parallel q/k/v projections, row-parallel out projection). Each core emits a
partial output [QL, DIM] (bf16); the host sums the 4 partials per batch in
fp32 and adds the output bias (the row-parallel all-reduce, at unshard time).

v2 layout (vs v1): phase-2 is software-pipelined at BLOCK granularity —
block n = (ig, head) covering 1024 queries; scores+exp of block n overlap
the PV matmuls of block n-1 (PV stream lagged LAG slots), so the scalar
engine (exp, ~130us total) hides under the tensor engine and the tensor
queue rarely idles (idle gaps reset the PE pstate ramp and halve the clock
for ~3us). Within-slot emission order alternates by parity so same-lhsT
matmul pairs abut (3 weight switches per 2 slots instead of 4; a weight
switch costs ~100ns on the first matmul using the new weights). The
v-projection is fused into block 0's score slots (each j-chunk accumulator
gets a PSUM bank to itself — accumulation-group state is per bank). All 16
out-proj units go into the drain, ordered so each final tail chain's
latency is covered by units whose attnT columns are already staged; the
last block's PV runs as two itp passes with half-tails for the same
reason. The scalar engine does nothing but exp in steady state.
"""

import numpy as np
import ml_dtypes

import concourse.bass as bass
import concourse.mybir as mybir
import concourse.tile as tile
from concourse import bacc
from concourse.bass_utils import run_bass_kernel_spmd

BF16 = mybir.dt.bfloat16
F32 = mybir.dt.float32

B = 2
DIM = 1024
NUM_HEADS = 16
HD = DIM // NUM_HEADS  # 64
SCALE = HD ** -0.5
NCORES = 8
NH = 4          # heads per core
CDIM = NH * HD  # 256, per-core slice of the head dim
P = 128
IT = 512        # i (query) tile
ECH = DIM // P  # 8 contraction chunks for the projections


def build_bass(QL=2048, KL=2048, num_devices=NCORES, dbg=False):
    assert QL == 2048 and KL == 2048
    NB = KL // P        # 16 j-chunks per block
    NIG = QL // (2 * IT)  # 2 i-groups of 1024 queries
    NBLK = NIG * NH     # 8 blocks

    nc = bacc.Bacc("TRN2", target_bir_lowering=False, debug=False,
                   num_devices=num_devices)
    if dbg:
        attnT_d = nc.dram_tensor("attnT_d", [P, 2, QL], BF16,
                                 kind="ExternalOutput").ap()
        vh_d = nc.dram_tensor("vh_d", [P, KL // P, NH, HD + 1], BF16,
                              kind="ExternalOutput").ap()
        qhT_d = nc.dram_tensor("qhT_d", [P, 2, QL], BF16,
                               kind="ExternalOutput").ap()
        khT_d = nc.dram_tensor("khT_d", [P, 2, KL], BF16,
                               kind="ExternalOutput").ap()
    qb = nc.dram_tensor("qbT", [DIM, QL], BF16, kind="ExternalInput").ap()
    kb = nc.dram_tensor("kbT", [DIM, KL], BF16, kind="ExternalInput").ap()
    vb = nc.dram_tensor("vbT", [DIM, KL], BF16, kind="ExternalInput").ap()
    wqT = nc.dram_tensor("wqT", [DIM, CDIM], BF16, kind="ExternalInput").ap()
    wkT = nc.dram_tensor("wkT", [DIM, CDIM], BF16, kind="ExternalInput").ap()
    wvT = nc.dram_tensor("wvT", [DIM, CDIM], BF16, kind="ExternalInput").ap()
    woT = nc.dram_tensor("woT", [CDIM, DIM], BF16, kind="ExternalInput").ap()
    outp = nc.dram_tensor("outp", [QL, DIM], BF16, kind="ExternalOutput").ap()

    with tile.TileContext(nc) as tc:
        with (
            tc.tile_pool(name="wpool", bufs=1) as wpool,
            tc.tile_pool(name="xpool", bufs=8) as xpool,
            tc.tile_pool(name="ppool", bufs=24) as ppool,
            tc.tile_pool(name="rpool", bufs=2) as rpool,
            tc.tile_pool(name="spool", bufs=2) as spool,
            tc.tile_pool(name="opool", bufs=4) as opool,
            tc.tile_pool(name="dpool", bufs=4, space="DRAM") as dpool,
            tc.tile_pool(name="psA", bufs=3, space="PSUM") as psA,   # 3x2 banks
            tc.tile_pool(name="psV", bufs=1, space="PSUM") as psV,   # 2 banks
        ):
            # ---- persistent SBUF tensors ----
            wq_sb = wpool.tile([P, ECH, CDIM], BF16, tag="wq")
            wk_sb = wpool.tile([P, ECH, CDIM], BF16, tag="wk")
            wv_sb = wpool.tile([P, ECH, CDIM], BF16, tag="wv")
            wo_sb = wpool.tile([P, CDIM // P, DIM], BF16, tag="wo")
            # chunked weight load: first matmul only needs chunk e=0
            for e in range(ECH):
                nc.gpsimd.dma_start(wq_sb[:, e, :], wqT[e * P:(e + 1) * P, :])

            qhT = wpool.tile([P, CDIM // P, QL], BF16, tag="qhT")
            khT = wpool.tile([P, CDIM // P, KL], BF16, tag="khT")
            vh = wpool.tile([P, NB, NH, HD + 1], BF16, tag="vh")
            attnT = wpool.tile([P, CDIM // P, QL], BF16, tag="attnT")
            nc.gpsimd.memset(vh[:, :, :, HD], 1.0)  # ones column -> row sums

            # ---- phase 1a/1b: q/k projections -> [d' part-major, token] ----
            # 4 two-bank accumulators (3 psA + 1 psV tiles); each matmul is
            # compound (one LDWEIGHTS + MATMUL x2 across the 2 banks).
            def proj_qk(x_dram, w_sb, dst, L, eng_pair, prefetch=None, pre=None):
                accs = [psA.tile([P, 2 * IT], F32, tag="psA", name=f"acc{i}")
                        for i in range(3)]
                accs.append(psV.tile([P, 2 * IT], F32, tag="psV", name="acc3"))
                for e in range(ECH):
                    if pre and e in pre:
                        xT = pre[e]
                    else:
                        xT = xpool.tile([P, L], BF16, tag="xT")
                        engs = (list(eng_pair) + [nc.scalar, nc.scalar]
                                if (e == 0 and pre is None) else list(eng_pair))
                        nq = len(engs)
                        for qq in range(nq):
                            engs[qq].dma_start(
                                xT[:, qq * (L // nq):(qq + 1) * (L // nq)],
                                x_dram[e * P:(e + 1) * P,
                                       qq * (L // nq):(qq + 1) * (L // nq)])
                    if prefetch and e == 5:
                        prefetch()
                    for d in range(2):
                        for it in range(L // IT):
                            nc.tensor.matmul(
                                accs[2 * d + it // 2][:, (it % 2) * IT:
                                                      (it % 2 + 1) * IT],
                                lhsT=w_sb[:, e, d * P:(d + 1) * P],
                                rhs=xT[:, it * IT:(it + 1) * IT],
                                start=(e == 0), stop=(e == ECH - 1))
                for d in range(2):
                    for ip in range(L // (2 * IT)):
                        dst_sl = dst[:, d, ip * 2 * IT:(ip + 1) * 2 * IT]
                        if ip % 2 == 0:
                            nc.scalar.copy(dst_sl, accs[2 * d + ip][:])
                        else:
                            nc.vector.tensor_copy(dst_sl, accs[2 * d + ip][:])

            # prefetch the next phase's first tiles on the scalar ring
            # (idle during the projections) so neither the k projection nor
            # the fusion phase starts on a DMA stall
            kpre, vpre = {}, {}

            def kprefetch():
                t = xpool.tile([P, KL], BF16, tag="xT", name="kxT0")
                for qq in range(2):
                    nc.scalar.dma_start(
                        t[:, qq * (KL // 2):(qq + 1) * (KL // 2)],
                        kb[0:P, qq * (KL // 2):(qq + 1) * (KL // 2)])
                kpre[0] = t
                for e in range(3):
                    nc.scalar.dma_start(wk_sb[:, e, :],
                                        wkT[e * P:(e + 1) * P, :])

            def vprefetch():
                for e in range(2):
                    t = xpool.tile([P, KL // 8], BF16, tag="xT",
                                   name=f"vxT{e}")
                    nc.scalar.dma_start(t[:], vb[e * P:(e + 1) * P, 0:KL // 8])
                    vpre[(0, e)] = t
                for e in range(3):
                    nc.scalar.dma_start(wv_sb[:, e, :],
                                        wvT[e * P:(e + 1) * P, :])

            proj_qk(qb, wq_sb, qhT, QL, (nc.sync, nc.gpsimd),
                    prefetch=kprefetch)
            for e in range(3, ECH):
                nc.gpsimd.dma_start(wk_sb[:, e, :], wkT[e * P:(e + 1) * P, :])
            proj_qk(kb, wk_sb, khT, KL, (nc.sync, nc.gpsimd), pre=kpre,
                    prefetch=vprefetch)
            for e in range(3, ECH):
                nc.gpsimd.dma_start(wv_sb[:, e, :], wvT[e * P:(e + 1) * P, :])
            nc.gpsimd.dma_start(wo_sb[:], woT.rearrange("(o p) d -> p o d", p=P))

            # ---- phase-2 machinery ----
            # Flat slot schedule: slot s carries scores(n=s//16, b=s%16),
            # the PV stream lagged by one block + LAG slots (so the tail of
            # block m drains its pv psum before block m+1's first PV matmul
            # reuses the single psV slot), and out-proj filler units.
            LAG = 4
            pt_live = {}   # (n, b) -> Pt tile
            pv_live = {}   # n -> pv psum tile

            def emit_scores(n, b):
                ig, h = n // NH, n % NH
                hp, hh = h // 2, h % 2
                k_h = khT[hh * HD:(hh + 1) * HD, hp, :]
                q_h = qhT[hh * HD:(hh + 1) * HD, hp, :]
                ps = psA.tile([P, 2 * IT], F32, tag="psA", name="s")
                for itp in range(2):
                    nc.tensor.matmul(
                        ps[:, itp * IT:(itp + 1) * IT],
                        lhsT=k_h[:, b * P:(b + 1) * P],
                        rhs=q_h[:, (ig * 2 + itp) * IT:(ig * 2 + itp + 1) * IT],
                        start=True, stop=True)
                Pt = ppool.tile([P, 2, IT], BF16, tag="Pt")
                nc.scalar.activation(
                    Pt[:], ps.rearrange("p (a b) -> p a b", a=2),
                    mybir.ActivationFunctionType.Exp, scale=SCALE)
                pt_live[(n, b)] = Pt

            def emit_pv(n, b):
                h = n % NH
                if b == 0:
                    pv_live[n] = psV.tile([P, 2 * IT], F32, tag="psV", name="pv")
                pv = pv_live[n]
                Pt = pt_live.pop((n, b))
                for itp in range(2):
                    nc.tensor.matmul(
                        pv[0:HD + 1, itp * IT:(itp + 1) * IT],
                        lhsT=vh[:, b, h, :], rhs=Pt[:, itp, :],
                        start=(b == 0), stop=(b == NB - 1))

            def emit_tail(n, itp=None):
                # itp=None: whole 1024-wide block tail. itp=0/1: half-tail
                # over 512 columns (used to split the final block's drain).
                ig, h = n // NH, n % NH
                hp, hh = h // 2, h % 2
                if itp is None:
                    pv_sl = pv_live.pop(n)[0:HD + 1, :]
                    c0, W = ig * 2 * IT, 2 * IT
                else:
                    pv_sl = pv_live[n][0:HD + 1, itp * IT:(itp + 1) * IT]
                    c0, W = (ig * 2 + itp) * IT, IT
                    if itp == 1:
                        pv_live.pop(n)
                # evacuate first: the copy (+ the sums DMA) free the psV
                # banks quickly so the next block's PV stream (LAG slots
                # behind) never stalls on the rest of this chain.
                st = spool.tile([HD + 1, 2 * IT], F32, tag="st", name="st")[:, 0:W]
                # two half-copies: the next block's first PV matmul (itp0
                # slice) only waits on the first half instead of the whole
                # 1.4us evacuation
                nc.vector.tensor_copy(st[:, 0:W // 2], pv_sl[:, 0:W // 2])
                nc.vector.tensor_copy(st[:, W // 2:W], pv_sl[:, W // 2:W])
                # sums row -> partition 0, then the custom-DVE reciprocal
                s0 = rpool.tile([1, 2 * IT], F32, tag="s0", name="s0")[:, 0:W]
                nc.sync.dma_start(s0[:], st[HD:HD + 1, :])
                rrec = rpool.tile([1, 2 * IT], F32, tag="rrec", name="rrec")[:, 0:W]
                rscr = rpool.tile([1, 2 * IT], F32, tag="rscr", name="rscr")[:, 0:W]
                nc.vector.reciprocal_approx_accurate(
                    out=rrec[:], in_=s0[:], scratch=rscr[:])
                dtmp = dpool.tile([1, 2 * IT], F32, tag="dtmp", name="dtmp")[:, 0:W]
                nc.sync.dma_start(dtmp[:], rrec[:])
                rbc = spool.tile([HD, 2 * IT], F32, tag="rbc", name="rbc")[:, 0:W]
                nc.sync.dma_start(
                    rbc[:], dtmp[0:1, :].broadcast_to((HD, W)))
                stb = spool.tile([HD, 2 * IT], BF16, tag="stb", name="stb")[:, 0:W]
                nc.vector.tensor_mul(stb[:], st[0:HD, :], rbc[:])
                nc.sync.dma_start(
                    attnT[hh * HD:(hh + 1) * HD, hp, c0:c0 + W], stb[:])

            uctr = [0]

            def emit_unit(icw, eng=None):
                # out-proj for queries icw*128..(icw+1)*128, all 1024 dims
                # (borrows a psA rotation slot; its evac frees it like an exp)
                po = psA.tile([P, 2 * IT], F32, tag="psA", name="po")
                for ec in range(CDIM // P):
                    for dt in range(2):
                        nc.tensor.matmul(
                            po[:, dt * IT:(dt + 1) * IT],
                            lhsT=attnT[:, ec, icw * P:(icw + 1) * P],
                            rhs=wo_sb[:, ec, dt * IT:(dt + 1) * IT],
                            start=(ec == 0), stop=(ec == CDIM // P - 1))
                ob = opool.tile([P, 2 * IT], BF16, tag="ob")
                if eng is None:
                    eng = nc.vector if uctr[0] % 2 == 0 else nc.scalar
                    uctr[0] += 1
                if eng is nc.scalar:
                    eng.copy(ob[:], po[:])
                else:
                    eng.tensor_copy(ob[:], po[:])
                # split the output write across two queues: DMA-issue
                # instructions cost ~650ns each and serialize per engine
                nc.gpsimd.dma_start(
                    outp[icw * P:icw * P + P // 2, :], ob[0:P // 2, :])
                nc.sync.dma_start(
                    outp[icw * P + P // 2:(icw + 1) * P, :], ob[P // 2:P, :])

            # out-proj units are NOT interleaved into the steady stream (it
            # is tensor-bound there; they would serialize). All 16 go into
            # the drain, ordered to cover the final tail-chain latencies.
            slot = [0]

            def pv_stream(s):
                m, b2 = divmod(s - NB - LAG, NB)
                if 0 <= m < NBLK - 1:
                    emit_pv(m, b2)
                    if b2 == NB - 1:
                        # pv(m) fully accumulated; drain it before block
                        # m+1's PV stream reuses the psV slot next slot.
                        emit_tail(m)

            # ---- phase 1c: v-projection fused with block 0's scores ----
            # v-proj: 8 rounds x 2 j-chunks; the 2 accumulators each get a
            # PSUM bank of the psV tile to themselves (accumulation-group
            # start/stop state is per bank) while block-0 scores rotate
            # through psA. Two score batches are emitted per round.
            for rnd in range(8):
                v0 = psV.tile([P, 2 * IT], F32, tag="psV", name="vaccV")
                vslots = [v0[:, 0:CDIM], v0[:, IT:IT + CDIM]]
                for e in range(ECH):
                    if (rnd, e) in vpre:
                        vT = vpre[(rnd, e)]
                    else:
                        vT = xpool.tile([P, KL // 8], BF16, tag="xT")
                        eng = nc.sync if e % 2 == 0 else nc.gpsimd
                        eng.dma_start(
                            vT[:], vb[e * P:(e + 1) * P,
                                      rnd * (KL // 8):(rnd + 1) * (KL // 8)])
                    for jc in range(2):
                        nc.tensor.matmul(vslots[jc], lhsT=vT[:, jc * P:(jc + 1) * P],
                                         rhs=wv_sb[:, e, :],
                                         start=(e == 0), stop=(e == ECH - 1))
                    if e % 4 == 0:
                        emit_scores(0, rnd * 2 + e // 4)
                        slot[0] += 1
                # evacuate the 2 j-chunk accumulators (alternating engines:
                # scalar has slack here, and the next round's first matmuls
                # wait on these copies)
                for jc in range(2):
                    j = rnd * 2 + jc
                    src = vslots[jc].rearrange("p (h c) -> p h c", h=NH)
                    if jc % 2 == 0:
                        nc.vector.tensor_copy(vh[:, j, :, 0:HD], src)
                    else:
                        nc.scalar.copy(vh[:, j, :, 0:HD], src)

            # ---- phase 2 steady state: scores blocks 1..7 ----
            # within-slot order alternates by parity (S,S,PV,PV | PV,PV,S,S)
            # so same-lhsT pairs abut across slot boundaries: 3 weight
            # switches per 2 slots instead of 4.
            for n in range(1, NBLK):
                for b in range(NB):
                    if slot[0] % 2 == 0:
                        emit_scores(n, b)
                        pv_stream(slot[0])
                    else:
                        pv_stream(slot[0])
                        emit_scores(n, b)
                    slot[0] += 1

            # ---- drain ----
            # finish the PV stream through block 6 + its tail
            while slot[0] <= NB * NBLK + LAG - 1:
                pv_stream(slot[0])
                slot[0] += 1
            # block 7's PV in two itp passes so its tail splits in half;
            # then the 16 out-proj units, ordered so each tail chain's
            # latency is covered by units whose columns are already staged:
            #   itp0 pass | itp1 pass + units 0..7 | units 8..11 | 12..15
            h7 = NBLK - 1
            pv7 = psV.tile([P, 2 * IT], F32, tag="psV", name="pv")
            for itp in range(2):
                for b in range(NB):
                    Pt = pt_live[(h7, b)] if itp == 0 else pt_live.pop((h7, b))
                    nc.tensor.matmul(
                        pv7[0:HD + 1, itp * IT:(itp + 1) * IT],
                        lhsT=vh[:, b, h7 % NH, :], rhs=Pt[:, itp, :],
                        start=(b == 0), stop=(b == NB - 1))
                pv_live[h7] = pv7
                emit_tail(h7, itp)
            for icw in list(range(8)) + list(range(8, 12)) + list(range(12, 16)):
                emit_unit(icw)

            if dbg:
                nc.sync.dma_start(attnT_d, attnT[:])
                nc.sync.dma_start(vh_d, vh[:])
                nc.sync.dma_start(qhT_d, qhT[:])
                nc.sync.dma_start(khT_d, khT[:])

    nc.compile()
    return nc


_NC_CACHE = {}


def _get_nc(QL, KL):
    key = (QL, KL)
    if key not in _NC_CACHE:
        _NC_CACHE[key] = build_bass(QL, KL)
    return _NC_CACHE[key]


def make_in_maps(q, k, v, Wq, Wk, Wv, Wo):
    """Per-core input maps (bf16, weights pre-transposed)."""
    bf = ml_dtypes.bfloat16
    q, k, v = (np.asarray(x, np.float32) for x in (q, k, v))
    WqT = np.asarray(Wq, np.float32).T.astype(bf)
    WkT = np.asarray(Wk, np.float32).T.astype(bf)
    WvT = np.asarray(Wv, np.float32).T.astype(bf)
    WoT = np.asarray(Wo, np.float32).T.astype(bf)
    qb = [np.ascontiguousarray(q[b].T.astype(bf)) for b in range(B)]
    kb = [np.ascontiguousarray(k[b].T.astype(bf)) for b in range(B)]
    vb = [np.ascontiguousarray(v[b].T.astype(bf)) for b in range(B)]
    in_maps = []
    for c in range(NCORES):
        b, hs = c // 4, c % 4
        sl = slice(hs * CDIM, (hs + 1) * CDIM)
        in_maps.append({
            "qbT": qb[b], "kbT": kb[b], "vbT": vb[b],
            "wqT": np.ascontiguousarray(WqT[:, sl]),
            "wkT": np.ascontiguousarray(WkT[:, sl]),
            "wvT": np.ascontiguousarray(WvT[:, sl]),
            "woT": np.ascontiguousarray(WoT[sl, :]),
        })
    return in_maps


def kernel(q, k, v, Wq, Wk, Wv, Wo, bo, _trace=False):
    q = np.asarray(q, np.float32)
    QL, KL = q.shape[1], np.asarray(k).shape[1]
    nc = _get_nc(QL, KL)
    in_maps = make_in_maps(q, k, v, Wq, Wk, Wv, Wo)
    res = run_bass_kernel_spmd(nc, in_maps, core_ids=list(range(NCORES)),
                               trace=_trace)
    bo = np.asarray(bo, np.float32)
    out = np.empty((B, QL, DIM), np.float32)
    for b in range(B):
        acc = res.results[4 * b]["outp"].astype(np.float32)
        for c in range(4 * b + 1, 4 * b + 4):
            acc += res.results[c]["outp"].astype(np.float32)
        out[b] = acc + bo
    if _trace:
        kernel._last_results = res
    return out


# revision 55
# speedup vs baseline: 1.0387x; 1.0387x over previous
"""Multi-head attention (B=2, QL=KL=2048, DIM=1024, H=16) on 8 TRN2 NeuronCores.

Sharding: core c handles batch c//4 and heads (c%4)*4 .. (c%4)*4+4 (column-
parallel q/k/v projections, row-parallel out projection). Each core emits a
partial output [QL, DIM] (bf16); the host sums the 4 partials per batch in
fp32 and adds the output bias (the row-parallel all-reduce, at unshard time).

v2 layout (vs v1): phase-2 is software-pipelined at BLOCK granularity —
block n = (ig, head) covering 1024 queries; scores+exp of block n overlap
the PV matmuls of block n-1 (PV stream lagged LAG slots), so the scalar
engine (exp, ~130us total) hides under the tensor engine and the tensor
queue rarely idles (idle gaps reset the PE pstate ramp and halve the clock
for ~3us). Within-slot emission order alternates by parity so same-lhsT
matmul pairs abut (3 weight switches per 2 slots instead of 4; a weight
switch costs ~100ns on the first matmul using the new weights). The
v-projection is fused into block 0's score slots (each j-chunk accumulator
gets a PSUM bank to itself — accumulation-group state is per bank). All 16
out-proj units go into the drain, ordered so each final tail chain's
latency is covered by units whose attnT columns are already staged; the
last block's PV runs as two itp passes with half-tails for the same
reason. The scalar engine does nothing but exp in steady state.
"""

import numpy as np
import ml_dtypes

import concourse.bass as bass
import concourse.mybir as mybir
import concourse.tile as tile
from concourse import bacc
from concourse.bass_utils import run_bass_kernel_spmd

BF16 = mybir.dt.bfloat16
F32 = mybir.dt.float32

B = 2
DIM = 1024
NUM_HEADS = 16
HD = DIM // NUM_HEADS  # 64
SCALE = HD ** -0.5
NCORES = 8
NH = 4          # heads per core
CDIM = NH * HD  # 256, per-core slice of the head dim
P = 128
IT = 512        # i (query) tile
ECH = DIM // P  # 8 contraction chunks for the projections


def build_bass(QL=2048, KL=2048, num_devices=NCORES, dbg=False):
    assert QL == 2048 and KL == 2048
    NB = KL // P        # 16 j-chunks per block
    NIG = QL // (2 * IT)  # 2 i-groups of 1024 queries
    NBLK = NIG * NH     # 8 blocks

    nc = bacc.Bacc("TRN2", target_bir_lowering=False, debug=False,
                   num_devices=num_devices)
    if dbg:
        attnT_d = nc.dram_tensor("attnT_d", [P, 2, QL], BF16,
                                 kind="ExternalOutput").ap()
        vh_d = nc.dram_tensor("vh_d", [P, KL // P, NH, HD + 1], BF16,
                              kind="ExternalOutput").ap()
        qhT_d = nc.dram_tensor("qhT_d", [P, 2, QL], BF16,
                               kind="ExternalOutput").ap()
        khT_d = nc.dram_tensor("khT_d", [P, 2, KL], BF16,
                               kind="ExternalOutput").ap()
    qb = nc.dram_tensor("qbT", [DIM, QL], BF16, kind="ExternalInput").ap()
    kb = nc.dram_tensor("kbT", [DIM, KL], BF16, kind="ExternalInput").ap()
    vb = nc.dram_tensor("vbT", [DIM, KL], BF16, kind="ExternalInput").ap()
    wqT = nc.dram_tensor("wqT", [DIM, CDIM], BF16, kind="ExternalInput").ap()
    wkT = nc.dram_tensor("wkT", [DIM, CDIM], BF16, kind="ExternalInput").ap()
    wvT = nc.dram_tensor("wvT", [DIM, CDIM], BF16, kind="ExternalInput").ap()
    woT = nc.dram_tensor("woT", [CDIM, DIM], BF16, kind="ExternalInput").ap()
    outp = nc.dram_tensor("outp", [QL, DIM], BF16, kind="ExternalOutput").ap()

    with tile.TileContext(nc) as tc:
        with (
            tc.tile_pool(name="wpool", bufs=1) as wpool,
            tc.tile_pool(name="xpool", bufs=8) as xpool,
            tc.tile_pool(name="ppool", bufs=24) as ppool,
            tc.tile_pool(name="rpool", bufs=2) as rpool,
            tc.tile_pool(name="spool", bufs=2) as spool,
            tc.tile_pool(name="opool", bufs=4) as opool,
            tc.tile_pool(name="dpool", bufs=4, space="DRAM") as dpool,
            tc.tile_pool(name="psA", bufs=3, space="PSUM") as psA,   # 3x2 banks
            tc.tile_pool(name="psV", bufs=1, space="PSUM") as psV,   # 2 banks
        ):
            # ---- persistent SBUF tensors ----
            wq_sb = wpool.tile([P, ECH, CDIM], BF16, tag="wq")
            wk_sb = wpool.tile([P, ECH, CDIM], BF16, tag="wk")
            wv_sb = wpool.tile([P, ECH, CDIM], BF16, tag="wv")
            wo_sb = wpool.tile([P, CDIM // P, DIM], BF16, tag="wo")
            # chunked weight load: first matmul only needs chunk e=0
            for e in range(ECH):
                nc.gpsimd.dma_start(wq_sb[:, e, :], wqT[e * P:(e + 1) * P, :])

            qhT = wpool.tile([P, CDIM // P, QL], BF16, tag="qhT")
            khT = wpool.tile([P, CDIM // P, KL], BF16, tag="khT")
            vh = wpool.tile([P, NB, NH, HD + 1], BF16, tag="vh")
            attnT = wpool.tile([P, CDIM // P, QL], BF16, tag="attnT")
            nc.gpsimd.memset(vh[:, :, :, HD], 1.0)  # ones column -> row sums

            # ---- phase 1a/1b: q/k projections -> [d' part-major, token] ----
            # 4 two-bank accumulators (3 psA + 1 psV tiles); each matmul is
            # compound (one LDWEIGHTS + MATMUL x2 across the 2 banks).
            def proj_qk(x_dram, w_sb, dst, L, eng_pair, prefetch=None, pre=None):
                accs = [psA.tile([P, 2 * IT], F32, tag="psA", name=f"acc{i}")
                        for i in range(3)]
                accs.append(psV.tile([P, 2 * IT], F32, tag="psV", name="acc3"))
                for e in range(ECH):
                    if pre and e in pre:
                        xT = pre[e]
                    else:
                        xT = xpool.tile([P, L], BF16, tag="xT")
                        engs = (list(eng_pair) + [nc.scalar, nc.scalar]
                                if (e == 0 and pre is None) else list(eng_pair))
                        nq = len(engs)
                        for qq in range(nq):
                            engs[qq].dma_start(
                                xT[:, qq * (L // nq):(qq + 1) * (L // nq)],
                                x_dram[e * P:(e + 1) * P,
                                       qq * (L // nq):(qq + 1) * (L // nq)])
                    if prefetch and e == 5:
                        prefetch()
                    for d in range(2):
                        for it in range(L // IT):
                            nc.tensor.matmul(
                                accs[2 * d + it // 2][:, (it % 2) * IT:
                                                      (it % 2 + 1) * IT],
                                lhsT=w_sb[:, e, d * P:(d + 1) * P],
                                rhs=xT[:, it * IT:(it + 1) * IT],
                                start=(e == 0), stop=(e == ECH - 1))
                for d in range(2):
                    for ip in range(L // (2 * IT)):
                        dst_sl = dst[:, d, ip * 2 * IT:(ip + 1) * 2 * IT]
                        if ip % 2 == 0:
                            nc.scalar.copy(dst_sl, accs[2 * d + ip][:])
                        else:
                            nc.vector.tensor_copy(dst_sl, accs[2 * d + ip][:])

            # prefetch the next phase's first tiles on the scalar ring
            # (idle during the projections) so neither the k projection nor
            # the fusion phase starts on a DMA stall
            kpre, vpre = {}, {}

            def kprefetch():
                t = xpool.tile([P, KL], BF16, tag="xT", name="kxT0")
                for qq in range(2):
                    nc.scalar.dma_start(
                        t[:, qq * (KL // 2):(qq + 1) * (KL // 2)],
                        kb[0:P, qq * (KL // 2):(qq + 1) * (KL // 2)])
                kpre[0] = t
                for e in range(3):
                    nc.scalar.dma_start(wk_sb[:, e, :],
                                        wkT[e * P:(e + 1) * P, :])

            def vprefetch():
                for e in range(2):
                    t = xpool.tile([P, KL // 8], BF16, tag="xT",
                                   name=f"vxT{e}")
                    nc.scalar.dma_start(t[:], vb[e * P:(e + 1) * P, 0:KL // 8])
                    vpre[(0, e)] = t
                for e in range(3):
                    nc.scalar.dma_start(wv_sb[:, e, :],
                                        wvT[e * P:(e + 1) * P, :])

            proj_qk(qb, wq_sb, qhT, QL, (nc.sync, nc.gpsimd),
                    prefetch=kprefetch)
            for e in range(3, ECH):
                nc.gpsimd.dma_start(wk_sb[:, e, :], wkT[e * P:(e + 1) * P, :])
            proj_qk(kb, wk_sb, khT, KL, (nc.sync, nc.gpsimd), pre=kpre,
                    prefetch=vprefetch)
            for e in range(3, ECH):
                nc.gpsimd.dma_start(wv_sb[:, e, :], wvT[e * P:(e + 1) * P, :])
            nc.gpsimd.dma_start(wo_sb[:], woT.rearrange("(o p) d -> p o d", p=P))

            # ---- phase-2 machinery ----
            # Flat slot schedule: slot s carries scores(n=s//16, b=s%16),
            # the PV stream lagged by one block + LAG slots (so the tail of
            # block m drains its pv psum before block m+1's first PV matmul
            # reuses the single psV slot), and out-proj filler units.
            LAG = 4
            pt_live = {}   # (n, b) -> Pt tile
            pv_live = {}   # n -> pv psum tile

            def emit_scores(n, b):
                ig, h = n // NH, n % NH
                hp, hh = h // 2, h % 2
                k_h = khT[hh * HD:(hh + 1) * HD, hp, :]
                q_h = qhT[hh * HD:(hh + 1) * HD, hp, :]
                ps = psA.tile([P, 2 * IT], F32, tag="psA", name="s")
                for itp in range(2):
                    nc.tensor.matmul(
                        ps[:, itp * IT:(itp + 1) * IT],
                        lhsT=k_h[:, b * P:(b + 1) * P],
                        rhs=q_h[:, (ig * 2 + itp) * IT:(ig * 2 + itp + 1) * IT],
                        start=True, stop=True)
                Pt = ppool.tile([P, 2, IT], BF16, tag="Pt")
                nc.scalar.activation(
                    Pt[:], ps.rearrange("p (a b) -> p a b", a=2),
                    mybir.ActivationFunctionType.Exp, scale=SCALE)
                pt_live[(n, b)] = Pt

            def emit_pv(n, b):
                h = n % NH
                if b == 0:
                    pv_live[n] = psV.tile([P, 2 * IT], F32, tag="psV", name="pv")
                pv = pv_live[n]
                Pt = pt_live.pop((n, b))
                for itp in range(2):
                    nc.tensor.matmul(
                        pv[0:HD + 1, itp * IT:(itp + 1) * IT],
                        lhsT=vh[:, b, h, :], rhs=Pt[:, itp, :],
                        start=(b == 0), stop=(b == NB - 1))

            def emit_tail(n, itp=None):
                # itp=None: whole 1024-wide block tail. itp=0/1: half-tail
                # over 512 columns (used to split the final block's drain).
                ig, h = n // NH, n % NH
                hp, hh = h // 2, h % 2
                if itp is None:
                    pv_sl = pv_live.pop(n)[0:HD + 1, :]
                    c0, W = ig * 2 * IT, 2 * IT
                else:
                    pv_sl = pv_live[n][0:HD + 1, itp * IT:(itp + 1) * IT]
                    c0, W = (ig * 2 + itp) * IT, IT
                    if itp == 1:
                        pv_live.pop(n)
                # evacuate first: the copy (+ the sums DMA) free the psV
                # banks quickly so the next block's PV stream (LAG slots
                # behind) never stalls on the rest of this chain.
                st = spool.tile([HD + 1, 2 * IT], F32, tag="st", name="st")[:, 0:W]
                nc.vector.tensor_copy(st[:], pv_sl)
                # sums row -> partition 0, then the custom-DVE reciprocal
                s0 = rpool.tile([1, 2 * IT], F32, tag="s0", name="s0")[:, 0:W]
                nc.sync.dma_start(s0[:], st[HD:HD + 1, :])
                rrec = rpool.tile([1, 2 * IT], F32, tag="rrec", name="rrec")[:, 0:W]
                rscr = rpool.tile([1, 2 * IT], F32, tag="rscr", name="rscr")[:, 0:W]
                nc.vector.reciprocal_approx_accurate(
                    out=rrec[:], in_=s0[:], scratch=rscr[:])
                dtmp = dpool.tile([1, 2 * IT], F32, tag="dtmp", name="dtmp")[:, 0:W]
                nc.sync.dma_start(dtmp[:], rrec[:])
                rbc = spool.tile([HD, 2 * IT], F32, tag="rbc", name="rbc")[:, 0:W]
                nc.sync.dma_start(
                    rbc[:], dtmp[0:1, :].broadcast_to((HD, W)))
                stb = spool.tile([HD, 2 * IT], BF16, tag="stb", name="stb")[:, 0:W]
                nc.vector.tensor_mul(stb[:], st[0:HD, :], rbc[:])
                nc.sync.dma_start(
                    attnT[hh * HD:(hh + 1) * HD, hp, c0:c0 + W], stb[:])

            uctr = [0]

            def emit_unit(icw, eng=None):
                # out-proj for queries icw*128..(icw+1)*128, all 1024 dims
                # (borrows a psA rotation slot; its evac frees it like an exp)
                po = psA.tile([P, 2 * IT], F32, tag="psA", name="po")
                for ec in range(CDIM // P):
                    for dt in range(2):
                        nc.tensor.matmul(
                            po[:, dt * IT:(dt + 1) * IT],
                            lhsT=attnT[:, ec, icw * P:(icw + 1) * P],
                            rhs=wo_sb[:, ec, dt * IT:(dt + 1) * IT],
                            start=(ec == 0), stop=(ec == CDIM // P - 1))
                ob = opool.tile([P, 2 * IT], BF16, tag="ob")
                if eng is None:
                    eng = nc.vector if uctr[0] % 2 == 0 else nc.scalar
                    uctr[0] += 1
                if eng is nc.scalar:
                    eng.copy(ob[:], po[:])
                else:
                    eng.tensor_copy(ob[:], po[:])
                # split the output write across two queues: DMA-issue
                # instructions cost ~650ns each and serialize per engine
                nc.gpsimd.dma_start(
                    outp[icw * P:icw * P + P // 2, :], ob[0:P // 2, :])
                nc.sync.dma_start(
                    outp[icw * P + P // 2:(icw + 1) * P, :], ob[P // 2:P, :])

            # out-proj units are NOT interleaved into the steady stream (it
            # is tensor-bound there; they would serialize). All 16 go into
            # the drain, ordered to cover the final tail-chain latencies.
            slot = [0]

            def pv_stream(s):
                m, b2 = divmod(s - NB - LAG, NB)
                if 0 <= m < NBLK - 1:
                    emit_pv(m, b2)
                    if b2 == NB - 1:
                        # pv(m) fully accumulated; drain it before block
                        # m+1's PV stream reuses the psV slot next slot.
                        emit_tail(m)

            # ---- phase 1c: v-projection fused with block 0's scores ----
            # v-proj: 8 rounds x 2 j-chunks; the 2 accumulators each get a
            # PSUM bank of the psV tile to themselves (accumulation-group
            # start/stop state is per bank) while block-0 scores rotate
            # through psA. Two score batches are emitted per round.
            for rnd in range(8):
                v0 = psV.tile([P, 2 * IT], F32, tag="psV", name="vaccV")
                vslots = [v0[:, 0:CDIM], v0[:, IT:IT + CDIM]]
                for e in range(ECH):
                    if (rnd, e) in vpre:
                        vT = vpre[(rnd, e)]
                    else:
                        vT = xpool.tile([P, KL // 8], BF16, tag="xT")
                        eng = nc.sync if e % 2 == 0 else nc.gpsimd
                        eng.dma_start(
                            vT[:], vb[e * P:(e + 1) * P,
                                      rnd * (KL // 8):(rnd + 1) * (KL // 8)])
                    for jc in range(2):
                        nc.tensor.matmul(vslots[jc], lhsT=vT[:, jc * P:(jc + 1) * P],
                                         rhs=wv_sb[:, e, :],
                                         start=(e == 0), stop=(e == ECH - 1))
                    if e % 4 == 0:
                        emit_scores(0, rnd * 2 + e // 4)
                        slot[0] += 1
                # evacuate the 2 j-chunk accumulators (alternating engines:
                # scalar has slack here, and the next round's first matmuls
                # wait on these copies)
                for jc in range(2):
                    j = rnd * 2 + jc
                    src = vslots[jc].rearrange("p (h c) -> p h c", h=NH)
                    if jc % 2 == 0:
                        nc.vector.tensor_copy(vh[:, j, :, 0:HD], src)
                    else:
                        nc.scalar.copy(vh[:, j, :, 0:HD], src)

            # ---- phase 2 steady state: scores blocks 1..7 ----
            # within-slot order alternates by parity (S,S,PV,PV | PV,PV,S,S)
            # so same-lhsT pairs abut across slot boundaries: 3 weight
            # switches per 2 slots instead of 4.
            for n in range(1, NBLK):
                for b in range(NB):
                    if slot[0] % 2 == 0:
                        emit_scores(n, b)
                        pv_stream(slot[0])
                    else:
                        pv_stream(slot[0])
                        emit_scores(n, b)
                    slot[0] += 1

            # ---- drain ----
            # finish the PV stream through block 6 + its tail
            while slot[0] <= NB * NBLK + LAG - 1:
                pv_stream(slot[0])
                slot[0] += 1
            # block 7's PV in two itp passes so its tail splits in half;
            # then the 16 out-proj units, ordered so each tail chain's
            # latency is covered by units whose columns are already staged:
            #   itp0 pass | itp1 pass + units 0..7 | units 8..11 | 12..15
            h7 = NBLK - 1
            pv7 = psV.tile([P, 2 * IT], F32, tag="psV", name="pv")
            for itp in range(2):
                for b in range(NB):
                    Pt = pt_live[(h7, b)] if itp == 0 else pt_live.pop((h7, b))
                    nc.tensor.matmul(
                        pv7[0:HD + 1, itp * IT:(itp + 1) * IT],
                        lhsT=vh[:, b, h7 % NH, :], rhs=Pt[:, itp, :],
                        start=(b == 0), stop=(b == NB - 1))
                pv_live[h7] = pv7
                emit_tail(h7, itp)
            for icw in list(range(8)) + list(range(8, 12)) + list(range(12, 16)):
                emit_unit(icw)

            if dbg:
                nc.sync.dma_start(attnT_d, attnT[:])
                nc.sync.dma_start(vh_d, vh[:])
                nc.sync.dma_start(qhT_d, qhT[:])
                nc.sync.dma_start(khT_d, khT[:])

    nc.compile()
    return nc


_NC_CACHE = {}


def _get_nc(QL, KL):
    key = (QL, KL)
    if key not in _NC_CACHE:
        _NC_CACHE[key] = build_bass(QL, KL)
    return _NC_CACHE[key]


def make_in_maps(q, k, v, Wq, Wk, Wv, Wo):
    """Per-core input maps (bf16, weights pre-transposed)."""
    bf = ml_dtypes.bfloat16
    q, k, v = (np.asarray(x, np.float32) for x in (q, k, v))
    WqT = np.asarray(Wq, np.float32).T.astype(bf)
    WkT = np.asarray(Wk, np.float32).T.astype(bf)
    WvT = np.asarray(Wv, np.float32).T.astype(bf)
    WoT = np.asarray(Wo, np.float32).T.astype(bf)
    qb = [np.ascontiguousarray(q[b].T.astype(bf)) for b in range(B)]
    kb = [np.ascontiguousarray(k[b].T.astype(bf)) for b in range(B)]
    vb = [np.ascontiguousarray(v[b].T.astype(bf)) for b in range(B)]
    in_maps = []
    for c in range(NCORES):
        b, hs = c // 4, c % 4
        sl = slice(hs * CDIM, (hs + 1) * CDIM)
        in_maps.append({
            "qbT": qb[b], "kbT": kb[b], "vbT": vb[b],
            "wqT": np.ascontiguousarray(WqT[:, sl]),
            "wkT": np.ascontiguousarray(WkT[:, sl]),
            "wvT": np.ascontiguousarray(WvT[:, sl]),
            "woT": np.ascontiguousarray(WoT[sl, :]),
        })
    return in_maps


def kernel(q, k, v, Wq, Wk, Wv, Wo, bo, _trace=False):
    q = np.asarray(q, np.float32)
    QL, KL = q.shape[1], np.asarray(k).shape[1]
    nc = _get_nc(QL, KL)
    in_maps = make_in_maps(q, k, v, Wq, Wk, Wv, Wo)
    res = run_bass_kernel_spmd(nc, in_maps, core_ids=list(range(NCORES)),
                               trace=_trace)
    bo = np.asarray(bo, np.float32)
    out = np.empty((B, QL, DIM), np.float32)
    for b in range(B):
        acc = res.results[4 * b]["outp"].astype(np.float32)
        for c in range(4 * b + 1, 4 * b + 4):
            acc += res.results[c]["outp"].astype(np.float32)
        out[b] = acc + bo
    if _trace:
        kernel._last_results = res
    return out


# revision 60
# speedup vs baseline: 1.0572x; 1.0178x over previous
"""Multi-head attention (B=2, QL=KL=2048, DIM=1024, H=16) on 8 TRN2 NeuronCores.

Sharding: core c handles batch c//4 and heads (c%4)*4 .. (c%4)*4+4 (column-
parallel q/k/v projections, row-parallel out projection). Each core emits a
partial output [QL, DIM] (bf16); the host sums the 4 partials per batch in
fp32 and adds the output bias (the row-parallel all-reduce, at unshard time).

v2 layout (vs v1): phase-2 is software-pipelined at BLOCK granularity —
block n = (ig, head) covering 1024 queries; scores+exp of block n overlap
the PV matmuls of block n-1 (PV stream lagged LAG slots), so the scalar
engine (exp, ~130us total) hides under the tensor engine and the tensor
queue rarely idles (idle gaps reset the PE pstate ramp and halve the clock
for ~3us). Within-slot emission order alternates by parity so same-lhsT
matmul pairs abut (3 weight switches per 2 slots instead of 4; a weight
switch costs ~100ns on the first matmul using the new weights). The
v-projection is fused into block 0's score slots (each j-chunk accumulator
gets a PSUM bank to itself — accumulation-group state is per bank). All 16
out-proj units go into the drain, ordered so each final tail chain's
latency is covered by units whose attnT columns are already staged; the
last block's PV runs as two itp passes with half-tails for the same
reason. The scalar engine does nothing but exp in steady state.
"""

import numpy as np
import ml_dtypes

import concourse.bass as bass
import concourse.mybir as mybir
import concourse.tile as tile
from concourse import bacc
from concourse.bass_utils import run_bass_kernel_spmd

BF16 = mybir.dt.bfloat16
F32 = mybir.dt.float32

B = 2
DIM = 1024
NUM_HEADS = 16
HD = DIM // NUM_HEADS  # 64
SCALE = HD ** -0.5
NCORES = 8
NH = 4          # heads per core
CDIM = NH * HD  # 256, per-core slice of the head dim
P = 128
IT = 512        # i (query) tile
ECH = DIM // P  # 8 contraction chunks for the projections


def build_bass(QL=2048, KL=2048, num_devices=NCORES, dbg=False):
    assert QL == 2048 and KL == 2048
    NB = KL // P        # 16 j-chunks per block
    NIG = QL // (2 * IT)  # 2 i-groups of 1024 queries
    NBLK = NIG * NH     # 8 blocks

    nc = bacc.Bacc("TRN2", target_bir_lowering=False, debug=False,
                   num_devices=num_devices)
    if dbg:
        attnT_d = nc.dram_tensor("attnT_d", [P, 2, QL], BF16,
                                 kind="ExternalOutput").ap()
        vh_d = nc.dram_tensor("vh_d", [P, KL // P, NH, HD + 1], BF16,
                              kind="ExternalOutput").ap()
        qhT_d = nc.dram_tensor("qhT_d", [P, 2, QL], BF16,
                               kind="ExternalOutput").ap()
        khT_d = nc.dram_tensor("khT_d", [P, 2, KL], BF16,
                               kind="ExternalOutput").ap()
    qb = nc.dram_tensor("qbT", [DIM, QL], BF16, kind="ExternalInput").ap()
    kb = nc.dram_tensor("kbT", [DIM, KL], BF16, kind="ExternalInput").ap()
    vb = nc.dram_tensor("vbT", [DIM, KL], BF16, kind="ExternalInput").ap()
    wqT = nc.dram_tensor("wqT", [DIM, CDIM], BF16, kind="ExternalInput").ap()
    wkT = nc.dram_tensor("wkT", [DIM, CDIM], BF16, kind="ExternalInput").ap()
    wvT = nc.dram_tensor("wvT", [DIM, CDIM], BF16, kind="ExternalInput").ap()
    woT = nc.dram_tensor("woT", [CDIM, DIM], BF16, kind="ExternalInput").ap()
    outp = nc.dram_tensor("outp", [QL, DIM], BF16, kind="ExternalOutput").ap()

    with tile.TileContext(nc) as tc:
        with (
            tc.tile_pool(name="wpool", bufs=1) as wpool,
            tc.tile_pool(name="xpool", bufs=8) as xpool,
            tc.tile_pool(name="ppool", bufs=24) as ppool,
            tc.tile_pool(name="rpool", bufs=2) as rpool,
            tc.tile_pool(name="spool", bufs=2) as spool,
            tc.tile_pool(name="opool", bufs=4) as opool,
            tc.tile_pool(name="dpool", bufs=4, space="DRAM") as dpool,
            tc.tile_pool(name="psA", bufs=3, space="PSUM") as psA,   # 3x2 banks
            tc.tile_pool(name="psV", bufs=1, space="PSUM") as psV,   # 2 banks
        ):
            # ---- persistent SBUF tensors ----
            wq_sb = wpool.tile([P, ECH, CDIM], BF16, tag="wq")
            wk_sb = wpool.tile([P, ECH, CDIM], BF16, tag="wk")
            wv_sb = wpool.tile([P, ECH, CDIM], BF16, tag="wv")
            wo_sb = wpool.tile([P, CDIM // P, DIM], BF16, tag="wo")
            # chunked weight load: the first matmul only needs chunk e=0;
            # the rest are issued after the first x-tile quarters so they
            # don't delay it in the ring queue
            nc.gpsimd.dma_start(wq_sb[:, 0, :], wqT[0:P, :])

            qhT = wpool.tile([P, CDIM // P, QL], BF16, tag="qhT")
            khT = wpool.tile([P, CDIM // P, KL], BF16, tag="khT")
            vh = wpool.tile([P, NB, NH, HD + 1], BF16, tag="vh")
            attnT = wpool.tile([P, CDIM // P, QL], BF16, tag="attnT")
            nc.gpsimd.memset(vh[:, :, :, HD], 1.0)  # ones column -> row sums

            # ---- phase 1a/1b: q/k projections -> [d' part-major, token] ----
            # 4 two-bank accumulators (3 psA + 1 psV tiles); each matmul is
            # compound (one LDWEIGHTS + MATMUL x2 across the 2 banks).
            def proj_qk(x_dram, w_sb, dst, L, eng_pair, prefetch=None,
                        pre=None, prefetch0=None):
                accs = [psA.tile([P, 2 * IT], F32, tag="psA", name=f"acc{i}")
                        for i in range(3)]
                accs.append(psV.tile([P, 2 * IT], F32, tag="psV", name="acc3"))
                for e in range(ECH):
                    if pre and e in pre:
                        xT = pre[e]
                    else:
                        xT = xpool.tile([P, L], BF16, tag="xT")
                        engs = (list(eng_pair) + [nc.scalar, nc.scalar]
                                if (e == 0 and pre is None) else list(eng_pair))
                        nq = len(engs)
                        for qq in range(nq):
                            engs[qq].dma_start(
                                xT[:, qq * (L // nq):(qq + 1) * (L // nq)],
                                x_dram[e * P:(e + 1) * P,
                                       qq * (L // nq):(qq + 1) * (L // nq)])
                    if prefetch0 and e == 0:
                        prefetch0()
                    if prefetch and e == 5:
                        prefetch()
                    for d in range(2):
                        for it in range(L // IT):
                            nc.tensor.matmul(
                                accs[2 * d + it // 2][:, (it % 2) * IT:
                                                      (it % 2 + 1) * IT],
                                lhsT=w_sb[:, e, d * P:(d + 1) * P],
                                rhs=xT[:, it * IT:(it + 1) * IT],
                                start=(e == 0), stop=(e == ECH - 1))
                for d in range(2):
                    for ip in range(L // (2 * IT)):
                        dst_sl = dst[:, d, ip * 2 * IT:(ip + 1) * 2 * IT]
                        if ip % 2 == 0:
                            nc.scalar.copy(dst_sl, accs[2 * d + ip][:])
                        else:
                            nc.vector.tensor_copy(dst_sl, accs[2 * d + ip][:])

            # prefetch the next phase's first tiles on the scalar ring
            # (idle during the projections) so neither the k projection nor
            # the fusion phase starts on a DMA stall
            kpre, vpre = {}, {}

            def kprefetch():
                t = xpool.tile([P, KL], BF16, tag="xT", name="kxT0")
                for qq in range(2):
                    nc.scalar.dma_start(
                        t[:, qq * (KL // 2):(qq + 1) * (KL // 2)],
                        kb[0:P, qq * (KL // 2):(qq + 1) * (KL // 2)])
                kpre[0] = t
                for e in range(3):
                    nc.scalar.dma_start(wk_sb[:, e, :],
                                        wkT[e * P:(e + 1) * P, :])

            def vprefetch():
                for e in range(2):
                    t = xpool.tile([P, KL // 8], BF16, tag="xT",
                                   name=f"vxT{e}")
                    nc.scalar.dma_start(t[:], vb[e * P:(e + 1) * P, 0:KL // 8])
                    vpre[(0, e)] = t
                for e in range(3):
                    nc.scalar.dma_start(wv_sb[:, e, :],
                                        wvT[e * P:(e + 1) * P, :])

            def wq_rest():
                for e in range(1, ECH):
                    nc.gpsimd.dma_start(wq_sb[:, e, :],
                                        wqT[e * P:(e + 1) * P, :])

            proj_qk(qb, wq_sb, qhT, QL, (nc.sync, nc.gpsimd),
                    prefetch=kprefetch, prefetch0=wq_rest)
            for e in range(3, ECH):
                nc.gpsimd.dma_start(wk_sb[:, e, :], wkT[e * P:(e + 1) * P, :])
            proj_qk(kb, wk_sb, khT, KL, (nc.sync, nc.gpsimd), pre=kpre,
                    prefetch=vprefetch)
            for e in range(3, ECH):
                nc.gpsimd.dma_start(wv_sb[:, e, :], wvT[e * P:(e + 1) * P, :])
            nc.gpsimd.dma_start(wo_sb[:], woT.rearrange("(o p) d -> p o d", p=P))

            # ---- phase-2 machinery ----
            # Flat slot schedule: slot s carries scores(n=s//16, b=s%16),
            # the PV stream lagged by one block + LAG slots (so the tail of
            # block m drains its pv psum before block m+1's first PV matmul
            # reuses the single psV slot), and out-proj filler units.
            LAG = 4
            pt_live = {}   # (n, b) -> Pt tile
            pv_live = {}   # n -> pv psum tile

            def emit_scores(n, b):
                ig, h = n // NH, n % NH
                hp, hh = h // 2, h % 2
                k_h = khT[hh * HD:(hh + 1) * HD, hp, :]
                q_h = qhT[hh * HD:(hh + 1) * HD, hp, :]
                ps = psA.tile([P, 2 * IT], F32, tag="psA", name="s")
                for itp in range(2):
                    nc.tensor.matmul(
                        ps[:, itp * IT:(itp + 1) * IT],
                        lhsT=k_h[:, b * P:(b + 1) * P],
                        rhs=q_h[:, (ig * 2 + itp) * IT:(ig * 2 + itp + 1) * IT],
                        start=True, stop=True)
                Pt = ppool.tile([P, 2, IT], BF16, tag="Pt")
                nc.scalar.activation(
                    Pt[:], ps.rearrange("p (a b) -> p a b", a=2),
                    mybir.ActivationFunctionType.Exp, scale=SCALE)
                pt_live[(n, b)] = Pt

            def emit_pv(n, b):
                h = n % NH
                if b == 0:
                    pv_live[n] = psV.tile([P, 2 * IT], F32, tag="psV", name="pv")
                pv = pv_live[n]
                Pt = pt_live.pop((n, b))
                for itp in range(2):
                    nc.tensor.matmul(
                        pv[0:HD + 1, itp * IT:(itp + 1) * IT],
                        lhsT=vh[:, b, h, :], rhs=Pt[:, itp, :],
                        start=(b == 0), stop=(b == NB - 1))

            def emit_tail(n, itp=None):
                # itp=None: whole 1024-wide block tail. itp=0/1: half-tail
                # over 512 columns (used to split the final block's drain).
                ig, h = n // NH, n % NH
                hp, hh = h // 2, h % 2
                if itp is None:
                    pv_sl = pv_live.pop(n)[0:HD + 1, :]
                    c0, W = ig * 2 * IT, 2 * IT
                else:
                    pv_sl = pv_live[n][0:HD + 1, itp * IT:(itp + 1) * IT]
                    c0, W = (ig * 2 + itp) * IT, IT
                    if itp == 1:
                        pv_live.pop(n)
                # evacuate first: the copy (+ the sums DMA) free the psV
                # banks quickly so the next block's PV stream (LAG slots
                # behind) never stalls on the rest of this chain.
                st = spool.tile([HD + 1, 2 * IT], F32, tag="st", name="st")[:, 0:W]
                nc.vector.tensor_copy(st[:], pv_sl)
                # sums row -> partition 0, then the custom-DVE reciprocal
                s0 = rpool.tile([1, 2 * IT], F32, tag="s0", name="s0")[:, 0:W]
                nc.sync.dma_start(s0[:], st[HD:HD + 1, :])
                rrec = rpool.tile([1, 2 * IT], F32, tag="rrec", name="rrec")[:, 0:W]
                rscr = rpool.tile([1, 2 * IT], F32, tag="rscr", name="rscr")[:, 0:W]
                nc.vector.reciprocal_approx_accurate(
                    out=rrec[:], in_=s0[:], scratch=rscr[:])
                dtmp = dpool.tile([1, 2 * IT], F32, tag="dtmp", name="dtmp")[:, 0:W]
                nc.sync.dma_start(dtmp[:], rrec[:])
                rbc = spool.tile([HD, 2 * IT], F32, tag="rbc", name="rbc")[:, 0:W]
                nc.sync.dma_start(
                    rbc[:], dtmp[0:1, :].broadcast_to((HD, W)))
                stb = spool.tile([HD, 2 * IT], BF16, tag="stb", name="stb")[:, 0:W]
                nc.vector.tensor_mul(stb[:], st[0:HD, :], rbc[:])
                nc.sync.dma_start(
                    attnT[hh * HD:(hh + 1) * HD, hp, c0:c0 + W], stb[:])

            uctr = [0]

            def emit_unit(icw, eng=None):
                # out-proj for queries icw*128..(icw+1)*128, all 1024 dims
                # (borrows a psA rotation slot; its evac frees it like an exp)
                po = psA.tile([P, 2 * IT], F32, tag="psA", name="po")
                for ec in range(CDIM // P):
                    for dt in range(2):
                        nc.tensor.matmul(
                            po[:, dt * IT:(dt + 1) * IT],
                            lhsT=attnT[:, ec, icw * P:(icw + 1) * P],
                            rhs=wo_sb[:, ec, dt * IT:(dt + 1) * IT],
                            start=(ec == 0), stop=(ec == CDIM // P - 1))
                ob = opool.tile([P, 2 * IT], BF16, tag="ob")
                if eng is None:
                    eng = nc.vector if uctr[0] % 2 == 0 else nc.scalar
                    uctr[0] += 1
                if eng is nc.scalar:
                    eng.copy(ob[:], po[:])
                else:
                    eng.tensor_copy(ob[:], po[:])
                # split the output write across two queues: DMA-issue
                # instructions cost ~650ns each and serialize per engine
                nc.gpsimd.dma_start(
                    outp[icw * P:icw * P + P // 2, :], ob[0:P // 2, :])
                nc.sync.dma_start(
                    outp[icw * P + P // 2:(icw + 1) * P, :], ob[P // 2:P, :])

            # out-proj units are NOT interleaved into the steady stream (it
            # is tensor-bound there; they would serialize). All 16 go into
            # the drain, ordered to cover the final tail-chain latencies.
            slot = [0]

            def pv_stream(s):
                m, b2 = divmod(s - NB - LAG, NB)
                if 0 <= m < NBLK - 1:
                    # batch 0 is deferred one slot and paired with batch 1
                    # so the previous tail's evacuation copy has a full
                    # extra slot to free the psV banks (no boundary stall)
                    if b2 == 0:
                        return
                    if b2 == 1:
                        emit_pv(m, 0)
                    emit_pv(m, b2)
                    if b2 == NB - 1:
                        # pv(m) fully accumulated; drain it before block
                        # m+1's PV stream reuses the psV slot next slot.
                        emit_tail(m)

            # ---- phase 1c: v-projection fused with block 0's scores ----
            # v-proj: 8 rounds x 2 j-chunks; the 2 accumulators each get a
            # PSUM bank of the psV tile to themselves (accumulation-group
            # start/stop state is per bank) while block-0 scores rotate
            # through psA. Two score batches are emitted per round.
            for rnd in range(8):
                v0 = psV.tile([P, 2 * IT], F32, tag="psV", name="vaccV")
                vslots = [v0[:, 0:CDIM], v0[:, IT:IT + CDIM]]
                for e in range(ECH):
                    if (rnd, e) in vpre:
                        vT = vpre[(rnd, e)]
                    else:
                        vT = xpool.tile([P, KL // 8], BF16, tag="xT")
                        eng = nc.sync if e % 2 == 0 else nc.gpsimd
                        eng.dma_start(
                            vT[:], vb[e * P:(e + 1) * P,
                                      rnd * (KL // 8):(rnd + 1) * (KL // 8)])
                    for jc in range(2):
                        nc.tensor.matmul(vslots[jc], lhsT=vT[:, jc * P:(jc + 1) * P],
                                         rhs=wv_sb[:, e, :],
                                         start=(e == 0), stop=(e == ECH - 1))
                    if e % 4 == 0:
                        emit_scores(0, rnd * 2 + e // 4)
                        slot[0] += 1
                # evacuate the 2 j-chunk accumulators (alternating engines:
                # scalar has slack here, and the next round's first matmuls
                # wait on these copies)
                for jc in range(2):
                    j = rnd * 2 + jc
                    src = vslots[jc].rearrange("p (h c) -> p h c", h=NH)
                    if jc % 2 == 0:
                        nc.vector.tensor_copy(vh[:, j, :, 0:HD], src)
                    else:
                        nc.scalar.copy(vh[:, j, :, 0:HD], src)

            # ---- phase 2 steady state: scores blocks 1..7 ----
            # within-slot order alternates by parity (S,S,PV,PV | PV,PV,S,S)
            # so same-lhsT pairs abut across slot boundaries: 3 weight
            # switches per 2 slots instead of 4.
            for n in range(1, NBLK):
                for b in range(NB):
                    if slot[0] % 2 == 0:
                        emit_scores(n, b)
                        pv_stream(slot[0])
                    else:
                        pv_stream(slot[0])
                        emit_scores(n, b)
                    slot[0] += 1

            # ---- drain ----
            # finish the PV stream through block 6 + its tail
            while slot[0] <= NB * NBLK + LAG - 1:
                pv_stream(slot[0])
                slot[0] += 1
            # block 7's PV in two itp passes so its tail splits in half;
            # then the 16 out-proj units, ordered so each tail chain's
            # latency is covered by units whose columns are already staged:
            #   itp0 pass | itp1 pass + units 0..7 | units 8..11 | 12..15
            h7 = NBLK - 1
            pv7 = psV.tile([P, 2 * IT], F32, tag="psV", name="pv")
            for itp in range(2):
                for b in range(NB):
                    Pt = pt_live[(h7, b)] if itp == 0 else pt_live.pop((h7, b))
                    nc.tensor.matmul(
                        pv7[0:HD + 1, itp * IT:(itp + 1) * IT],
                        lhsT=vh[:, b, h7 % NH, :], rhs=Pt[:, itp, :],
                        start=(b == 0), stop=(b == NB - 1))
                pv_live[h7] = pv7
                emit_tail(h7, itp)
            for icw in list(range(8)) + list(range(8, 12)) + list(range(12, 16)):
                emit_unit(icw)

            if dbg:
                nc.sync.dma_start(attnT_d, attnT[:])
                nc.sync.dma_start(vh_d, vh[:])
                nc.sync.dma_start(qhT_d, qhT[:])
                nc.sync.dma_start(khT_d, khT[:])

    nc.compile()
    return nc


_NC_CACHE = {}


def _get_nc(QL, KL):
    key = (QL, KL)
    if key not in _NC_CACHE:
        _NC_CACHE[key] = build_bass(QL, KL)
    return _NC_CACHE[key]


def make_in_maps(q, k, v, Wq, Wk, Wv, Wo):
    """Per-core input maps (bf16, weights pre-transposed)."""
    bf = ml_dtypes.bfloat16
    q, k, v = (np.asarray(x, np.float32) for x in (q, k, v))
    WqT = np.asarray(Wq, np.float32).T.astype(bf)
    WkT = np.asarray(Wk, np.float32).T.astype(bf)
    WvT = np.asarray(Wv, np.float32).T.astype(bf)
    WoT = np.asarray(Wo, np.float32).T.astype(bf)
    qb = [np.ascontiguousarray(q[b].T.astype(bf)) for b in range(B)]
    kb = [np.ascontiguousarray(k[b].T.astype(bf)) for b in range(B)]
    vb = [np.ascontiguousarray(v[b].T.astype(bf)) for b in range(B)]
    in_maps = []
    for c in range(NCORES):
        b, hs = c // 4, c % 4
        sl = slice(hs * CDIM, (hs + 1) * CDIM)
        in_maps.append({
            "qbT": qb[b], "kbT": kb[b], "vbT": vb[b],
            "wqT": np.ascontiguousarray(WqT[:, sl]),
            "wkT": np.ascontiguousarray(WkT[:, sl]),
            "wvT": np.ascontiguousarray(WvT[:, sl]),
            "woT": np.ascontiguousarray(WoT[sl, :]),
        })
    return in_maps


def kernel(q, k, v, Wq, Wk, Wv, Wo, bo, _trace=False):
    q = np.asarray(q, np.float32)
    QL, KL = q.shape[1], np.asarray(k).shape[1]
    nc = _get_nc(QL, KL)
    in_maps = make_in_maps(q, k, v, Wq, Wk, Wv, Wo)
    res = run_bass_kernel_spmd(nc, in_maps, core_ids=list(range(NCORES)),
                               trace=_trace)
    bo = np.asarray(bo, np.float32)
    out = np.empty((B, QL, DIM), np.float32)
    for b in range(B):
        acc = res.results[4 * b]["outp"].astype(np.float32)
        for c in range(4 * b + 1, 4 * b + 4):
            acc += res.results[c]["outp"].astype(np.float32)
        out[b] = acc + bo
    if _trace:
        kernel._last_results = res
    return out
